# revision 1
# baseline (speedup 1.0000x reference)
"""CosyVoice2 attention (B=8, S=2048, H=896, 14Q/2KV GQA, RoPE, causal) as a
Trainium2 Bass/Tile kernel, data-parallel over batch across 8 NeuronCores.

Per-core program (one batch element per core, no collectives):
  - host supplies X^T [896, 2048] (hidden on partitions, 7 chunks of 128),
    weights in matmul-ready layouts, and RoPE cos/sin tables with the
    rotate-half sign folded in.
  - QKV projections and o_proj run as fp32r matmuls (full PE rate at N>=256)
    with biases added by K=1 ones-row matmuls into the same PSUM group.
  - RoPE in [d, s] layout:  out = x.*cos4 + Pneg @ (x.*sinm4), where
    Pneg[i, i^32] = -1.  Since sinm4 alternates sign per 32-block,
    Pneg @ (x.*sinm4) == rot_half(x).*sinm4, so no cross-partition DVE ops.
  - scores S^T[k, q] per (head-pair, k-chunk): two K=64 matmuls row-tiled to
    opposite PE array halves run concurrently (heads h and h+7).  A second
    K^T copy with swapped kv halves (from a host-swapped Wk) provides the
    right partition alignment for odd pairs.
  - softmax: one ACT exp per k-chunk over the 2-bank PSUM pair
    ([128, 2, 512]), scale=1/8 and a constant -4 bias folded in; causal
    masking via one gpsimd affine_select on diagonal chunks only; the
    denominator rides as a 65th ones-row in the attnV lhsT.
  - normalize: evict denom row -> K=1 matmul broadcast -> fast reciprocal ->
    DVE multiply writing the A^T slab (odd-pair heads via cross-partition
    write at base 64; Wo rows are host-permuted to match).
"""

import os
import sys

for _p in ("/opt/trn_rl_repo", "/root/.axon_site/_ro/trn_rl_repo"):
    if _p not in sys.path and os.path.isdir(_p):
        sys.path.append(_p)

import contextlib

import numpy as np

import concourse.bacc as bacc
import concourse.mybir as mybir
import concourse.tile as tile
from concourse import bass_utils

B = 8
S = 2048
H = 896
NQ = 14
NKV = 2
D = 64
THETA = 1000000.0
P = 128
HC = H // P          # 7 hidden chunks
QT = 512             # q-tile width
NQT = S // QT        # 4 q-tiles
SC = S // P          # 16 seq chunks of 128
F32 = mybir.dt.float32
F32R = mybir.dt.float32r

_CACHE = {}
LAST_RESULTS = None


def _build():
    nc = bacc.Bacc("TRN2", target_bir_lowering=False, debug=False, num_devices=8)

    xt_d = nc.dram_tensor("xt", [P, HC, S], F32R, kind="ExternalInput").ap()
    wq_d = nc.dram_tensor("wq", [P, HC, H], F32R, kind="ExternalInput").ap()
    wk_d = nc.dram_tensor("wk", [P, HC, P], F32R, kind="ExternalInput").ap()
    wk2_d = nc.dram_tensor("wk2", [P, HC, P], F32R, kind="ExternalInput").ap()
    wv_d = nc.dram_tensor("wv", [P, HC, P], F32R, kind="ExternalInput").ap()
    wo_d = nc.dram_tensor("wo", [P, HC, H], F32R, kind="ExternalInput").ap()
    bq_d = nc.dram_tensor("bq", [1, H], F32R, kind="ExternalInput").ap()
    bk_d = nc.dram_tensor("bk", [1, P], F32R, kind="ExternalInput").ap()
    bk2_d = nc.dram_tensor("bk2", [1, P], F32R, kind="ExternalInput").ap()
    bv_d = nc.dram_tensor("bv", [1, P], F32R, kind="ExternalInput").ap()
    cos_d = nc.dram_tensor("cos4", [P, S], F32R, kind="ExternalInput").ap()
    sin_d = nc.dram_tensor("sinm4", [P, S], F32R, kind="ExternalInput").ap()
    pneg_d = nc.dram_tensor("pneg", [P, P], F32R, kind="ExternalInput").ap()
    ones_d = nc.dram_tensor("onesr", [P, QT], F32R, kind="ExternalInput").ap()
    ident_d = nc.dram_tensor("ident", [P, P], F32R, kind="ExternalInput").ap()
    o_d = nc.dram_tensor("o", [P, SC, H], F32, kind="ExternalOutput").ap()
    dbg = os.environ.get("KERNEL_DEBUG", "0") == "1"
    if dbg:
        dbg_kt = nc.dram_tensor("dbg_kt", [P, S], F32, kind="ExternalOutput").ap()
        dbg_kt2 = nc.dram_tensor("dbg_kt2", [P, S], F32, kind="ExternalOutput").ap()
        dbg_vp = nc.dram_tensor("dbg_vp", [P, SC, 130], F32, kind="ExternalOutput").ap()
        dbg_qs0 = nc.dram_tensor("dbg_qs0", [P, HC, QT], F32, kind="ExternalOutput").ap()
        dbg_as0 = nc.dram_tensor("dbg_as0", [P, HC, QT], F32, kind="ExternalOutput").ap()
        dbg_pr = nc.dram_tensor("dbg_pr", [P, 2, QT], F32, kind="ExternalOutput").ap()
        dbg_av = nc.dram_tensor("dbg_av", [65, 2, QT], F32, kind="ExternalOutput").ap()

    with tile.TileContext(nc) as tc, contextlib.ExitStack() as ctx:
        const = ctx.enter_context(tc.tile_pool(name="const", bufs=1))
        work = ctx.enter_context(tc.tile_pool(name="work", bufs=2))
        ppool = ctx.enter_context(tc.tile_pool(name="ppool", bufs=2))
        rpool = ctx.enter_context(tc.tile_pool(name="rpool", bufs=2))
        npool = ctx.enter_context(tc.tile_pool(name="npool", bufs=3))
        psc = ctx.enter_context(tc.tile_pool(name="psc", bufs=2, space="PSUM"))
        pssc = ctx.enter_context(tc.tile_pool(name="pssc", bufs=2, space="PSUM"))
        psav = ctx.enter_context(tc.tile_pool(name="psav", bufs=1, space="PSUM"))

        # ---- resident constants ----
        wk_sb = const.tile([P, HC, P], F32R)
        wk2_sb = const.tile([P, HC, P], F32R)
        wv_sb = const.tile([P, HC, P], F32R)
        wo_sb = const.tile([P, HC, H], F32R)
        bq_sb = const.tile([1, H], F32R)
        bk_sb = const.tile([1, P], F32R)
        bk2_sb = const.tile([1, P], F32R)
        bv_sb = const.tile([1, P], F32R)
        pneg_sb = const.tile([P, P], F32R)
        ident_sb = const.tile([P, P], F32R)
        ones_sb = const.tile([P, QT], F32R)
        bias_exp = const.tile([P, 1], F32)
        for dst, src in ((wk_sb, wk_d), (wk2_sb, wk2_d),
                         (wv_sb, wv_d), (wo_sb, wo_d), (bq_sb, bq_d),
                         (bk_sb, bk_d), (bk2_sb, bk2_d), (bv_sb, bv_d),
                         (pneg_sb, pneg_d), (ident_sb, ident_d),
                         (ones_sb, ones_d)):
            nc.sync.dma_start(out=dst, in_=src)
        nc.vector.memset(bias_exp, -4.0)

        # K^T resident (two partition layouts) and V' resident
        kt = const.tile([P, S], F32R)    # parts 0-63 = kv0, 64-127 = kv1
        kt2 = const.tile([P, S], F32R)   # parts 0-63 = kv1, 64-127 = kv0
        vp = const.tile([P, SC, 130], F32R)  # [Vkv0 | ones | Vkv1 | ones]
        nc.vector.memset(vp[:, :, 64:65].bitcast(F32), 1.0)
        nc.vector.memset(vp[:, :, 129:130].bitcast(F32), 1.0)

        # absorb weight-DMA waits with tiny PE touches (fp32 matmuls only
        # take one sync wait at the walrus level; make weight waits land
        # on these instead of real matmuls)
        tch = psc.tile([1, 2], F32, tag="proj", name="tch")
        for t in (wk_sb, wk2_sb, wv_sb, wo_sb,
                  pneg_sb, ident_sb, ones_sb):
            ap = (t[0:1, 0, 0:1] if len(t.shape) == 3 else t[0:1, 0:1]).bitcast(F32)
            nc.tensor.matmul(tch[:, 0:1], ap, ap, start=True, stop=True)
        for t in (bq_sb, bk_sb, bk2_sb, bv_sb):
            ap = t[0:1, 0:1].bitcast(F32)
            nc.tensor.matmul(tch[:, 0:1], ap, ap, start=True, stop=True)

        def rope_into(dst_ap, src_psum, cos_t, sin_t, nm):
            """dst = src*cos4 + Pneg @ (src*sinm4); evicts psum with 1 read"""
            qe = rpool.tile([P, QT], F32R, tag="qe", name=f"qe_{nm}")
            nc.vector.tensor_copy(qe, src_psum)
            t1 = rpool.tile([P, QT], F32R, tag="t1", name=f"t1_{nm}")
            nc.vector.tensor_mul(t1, qe, cos_t)
            nc.vector.tensor_mul(qe, qe, sin_t)
            rp = psc.tile([P, QT], F32, tag="proj", name=f"rp_{nm}")
            nc.tensor.matmul(rp, pneg_sb, qe, start=True, stop=True)
            nc.vector.tensor_add(dst_ap, t1.bitcast(F32), rp)

        state = {}

        def gen_proj(t):
            tslice = slice(t * QT, (t + 1) * QT)
            xs = work.tile([P, HC, QT], F32R, tag="xs", name=f"xs{t}")
            nc.sync.dma_start(out=xs, in_=xt_d[:, :, tslice])
            cos_t = work.tile([P, QT], F32R, tag="cos_t", name=f"cos{t}")
            sin_t = work.tile([P, QT], F32R, tag="sin_t", name=f"sin{t}")
            nc.sync.dma_start(out=cos_t, in_=cos_d[:, tslice])
            nc.sync.dma_start(out=sin_t, in_=sin_d[:, tslice])
            qs = work.tile([P, HC, QT], F32R, tag="qs", name=f"qs{t}")
            state[t] = {"qs": qs}
            # K projections (+swapped copy) + rope
            for kdst, w_sb, b_sb, nm in ((kt, wk_sb, bk_sb, f"k{t}"),
                                         (kt2, wk2_sb, bk2_sb, f"k2{t}")):
                kp = psc.tile([P, QT], F32, tag="proj", name=f"kp_{nm}")
                for c in range(HC):
                    nc.tensor.matmul(kp, w_sb[:, c, :], xs[:, c, :],
                                     start=(c == 0), stop=False)
                nc.tensor.matmul(kp, b_sb, ones_sb[0:1, :], start=False,
                                 stop=True)
                rope_into(kdst[:, tslice], kp, cos_t, sin_t, nm)
                yield
            # V projection: V^T then PE-transpose per 128-chunk
            vtp = psc.tile([P, QT], F32, tag="proj", name=f"vtp{t}")
            for c in range(HC):
                nc.tensor.matmul(vtp, wv_sb[:, c, :], xs[:, c, :],
                                 start=(c == 0), stop=False)
            nc.tensor.matmul(vtp, bv_sb, ones_sb[0:1, :], start=False,
                             stop=True)
            vt_sb = rpool.tile([P, QT], F32R, tag="vt_sb", name=f"vt{t}")
            nc.vector.tensor_copy(vt_sb, vtp)
            for j in range(4):
                sc_i = t * 4 + j
                vtr = psc.tile([P, P], F32R, tag="proj", name=f"vtr{sc_i}")
                nc.tensor.transpose(vtr, vt_sb[:, j * P:(j + 1) * P], ident_sb)
                nc.vector.tensor_copy(vp[:, sc_i, 0:64], vtr[:, 0:64])
                nc.vector.tensor_copy(vp[:, sc_i, 65:129], vtr[:, 64:128])
            yield
            # Q projection + rope (7 dq chunks)
            for c in range(HC):
                wq_c = work.tile([P, HC, P], F32R, tag="wq_c", bufs=1, name=f"wq{t}_{c}")
                nc.sync.dma_start(out=wq_c, in_=wq_d[:, :, c * P:(c + 1) * P])
                qp = psc.tile([P, QT], F32, tag="proj", name=f"qp{t}_{c}")
                for hcc in range(HC):
                    nc.tensor.matmul(qp, wq_c[:, hcc, :], xs[:, hcc, :],
                                     start=(hcc == 0), stop=False)
                nc.tensor.matmul(qp, bq_sb[:, c * P:(c + 1) * P],
                                 ones_sb[0:1, :], start=False, stop=True)
                rope_into(qs[:, c, :], qp, cos_t, sin_t, f"q{t}_{c}")
                yield

        def gen_oproj(t):
            aslab = state[t]["aslab"]
            for j in range(4):
                sc_i = t * 4 + j
                jsl = slice(j * P, (j + 1) * P)
                for n0, nw in ((0, 512), (512, 384)):
                    op = psc.tile([P, 512], F32, tag="proj",
                                  name=f"op{sc_i}_{n0}")
                    for c in range(HC):
                        nc.tensor.matmul(op[:, 0:nw], aslab[:, c, jsl],
                                         wo_sb[:, c, n0:n0 + nw],
                                         start=(c == 0), stop=(c == HC - 1))
                    osb = npool.tile([P, 512], F32, tag="osb", bufs=2,
                                     name=f"os{sc_i}_{n0}")
                    nc.vector.tensor_copy(osb[:, 0:nw], op[:, 0:nw])
                    nc.sync.dma_start(out=o_d[:, sc_i, n0:n0 + nw],
                                      in_=osb[:, 0:nw])
                    yield

        def attention_pair(t, hp):
            qs = state[t]["qs"]
            aslab = state[t]["aslab"]
            nkc = (t + 1) * 4
            h0, h1 = hp, hp + 7
            c0, r0 = h0 // 2, (h0 % 2) * 64
            c1, r1 = h1 // 2, (h1 % 2) * 64
            kt_h0 = kt if r0 == 0 else kt2
            kt_h1 = kt if r1 == 64 else kt2
            av0 = psav.tile([65, QT], F32, tag="av0", name=f"av0_{t}_{hp}")
            av1 = psav.tile([65, QT], F32, tag="av1", name=f"av1_{t}_{hp}")
            for kc in range(nkc):
                ksl = slice(kc * P, (kc + 1) * P)
                st = pssc.tile([P, 2, QT], F32, tag="st",
                               name=f"st{t}_{hp}_{kc}")
                nc.tensor.matmul(st[:, 0, :], kt_h0[r0:r0 + 64, ksl],
                                 qs[r0:r0 + 64, c0, :], start=True, stop=True)
                if r0 == 0:
                    nc.tensor.matmul(st[:, 1, :], kt_h1[64:128, ksl],
                                     qs[64:128, c1, :], start=True, stop=True,
                                     tile_position=(64, 0))
                else:
                    nc.tensor.matmul(st[:, 1, :], kt_h1[0:64, ksl],
                                     qs[0:64, c1, :], start=True, stop=True)
                probs = ppool.tile([P, 2, QT], F32R, tag="probs",
                                   name=f"pr{t}_{hp}_{kc}")
                nc.scalar.activation(probs, st,
                                     mybir.ActivationFunctionType.Exp,
                                     bias=bias_exp, scale=0.125)
                if kc >= 4 * t:  # diagonal chunk: causal mask
                    nc.gpsimd.affine_select(
                        out=probs, in_=probs, pattern=[[0, 2], [1, QT]],
                        compare_op=mybir.AluOpType.is_ge, fill=0.0,
                        base=t * QT - kc * P, channel_multiplier=-1)
                if dbg and t == 0 and hp == 0 and kc == 0:
                    nc.sync.dma_start(out=dbg_pr, in_=probs.bitcast(F32))
                nc.tensor.matmul(av0, vp[:, kc, 0:65], probs[:, 0, :],
                                 start=(kc == 0), stop=(kc == nkc - 1))
                nc.tensor.matmul(av1, vp[:, kc, 65:130], probs[:, 1, :],
                                 start=(kc == 0), stop=(kc == nkc - 1))
            if dbg and t == 0 and hp == 0:
                avs_dbg = npool.tile([65, 2, QT], F32, tag="avdbg", bufs=1,
                                     name="avdbg")
                nc.vector.tensor_copy(avs_dbg[:, 0, :], av0)
                nc.vector.tensor_copy(avs_dbg[:, 1, :], av1)
                nc.sync.dma_start(out=dbg_av, in_=avs_dbg)
            # normalize + write A^T slab
            for av, rh in ((av0, 0), (av1, 64)):
                nm = f"n{t}_{hp}_{rh}"
                dsb = npool.tile([P, QT], F32R, tag="dsb", bufs=2, name=f"d{nm}")
                nc.vector.tensor_copy(dsb[64:65, :], av[64:65, :])
                bc = pssc.tile([64, QT], F32, tag="st", name=f"bc{nm}")
                nc.tensor.matmul(bc, ones_sb[64:65, 0:64], dsb[64:65, :],
                                 start=True, stop=True)
                rc = npool.tile([64, QT], F32, tag="rc", bufs=2, name=f"r{nm}")
                nc.vector.reciprocal_approx_fast(rc, bc)
                nc.vector.tensor_mul(aslab[rh:rh + 64, hp, :],
                                     av[0:64, :], rc)

        # ---- software-pipelined emission ----
        for _ in gen_proj(0):
            pass
        for t in range(NQT):
            state[t]["aslab"] = work.tile([P, HC, QT], F32R, tag="aslab",
                                          name=f"aslab{t}")
            gnext = gen_proj(t + 1) if t + 1 < NQT else None
            gop = gen_oproj(t - 1) if t >= 1 else None
            for hp in range(7):
                attention_pair(t, hp)
                for g in (gnext, gnext, gop, gop):
                    if g is not None:
                        next(g, None)
            for g in (gnext, gop):
                if g is not None:
                    for _ in g:
                        pass
            if dbg and t == 0:
                nc.sync.dma_start(out=dbg_qs0, in_=state[0]["qs"].bitcast(F32))
                nc.sync.dma_start(out=dbg_as0,
                                  in_=state[0]["aslab"].bitcast(F32))
        for _ in gen_oproj(NQT - 1):
            pass
        if dbg:
            nc.sync.dma_start(out=dbg_kt, in_=kt.bitcast(F32))
            nc.sync.dma_start(out=dbg_kt2, in_=kt2.bitcast(F32))
            nc.sync.dma_start(out=dbg_vp, in_=vp.bitcast(F32))
    nc.compile()
    return nc


def _host_prep(hidden_states, position_ids, Wq, bq, Wk, bk, Wv, bv, Wo):
    """Build per-core input maps (host-side layout work)."""
    f32 = np.float32
    HALF = 32

    def chunked(w):  # [H, N] -> [P, HC, N]
        return np.ascontiguousarray(
            w.reshape(HC, P, -1).transpose(1, 0, 2)).astype(f32)

    wq_h = chunked(Wq)
    wk_h = chunked(Wk)
    swap = np.concatenate([np.arange(64, 128), np.arange(0, 64)])
    wk2_h = chunked(Wk[:, swap])
    wv_h = chunked(Wv)
    # Wo rows permuted: chunk hp partition p -> head (hp | hp+7), dim p%64
    perm = np.empty(H, np.int64)
    for hp in range(7):
        for p in range(P):
            h = hp if p < 64 else hp + 7
            perm[hp * P + p] = h * 64 + (p % 64)
    wo_h = chunked(Wo[perm])
    bq_h = bq.reshape(1, H).astype(f32)
    bk_h = bk.reshape(1, P).astype(f32)
    bk2_h = bk[swap].reshape(1, P).astype(f32)
    bv_h = bv.reshape(1, P).astype(f32)
    pneg = np.zeros((P, P), f32)
    for i in range(P):
        pneg[i, i ^ 32] = -1.0
    ones_h = np.ones((P, QT), f32)
    ident_h = np.eye(P, dtype=f32)

    inv_freq = (1.0 / (THETA ** (np.arange(0, HALF, dtype=np.float64) / HALF)))
    pidx = np.arange(P)
    sign = np.where((pidx % 64) >= HALF, 1.0, -1.0)[:, None]

    in_maps = []
    for b in range(B):
        xt = np.ascontiguousarray(
            hidden_states[b].T.reshape(HC, P, S).transpose(1, 0, 2)).astype(f32)
        ang = position_ids[b].astype(np.float64)[None, :] * \
            inv_freq[pidx % HALF][:, None]          # [P, S]
        cos4 = np.cos(ang).astype(f32)
        sinm4 = (np.sin(ang) * sign).astype(f32)
        in_maps.append({
            "xt": xt, "wq": wq_h, "wk": wk_h, "wk2": wk2_h, "wv": wv_h,
            "wo": wo_h, "bq": bq_h, "bk": bk_h, "bk2": bk2_h, "bv": bv_h,
            "cos4": cos4, "sinm4": sinm4, "pneg": pneg, "onesr": ones_h,
            "ident": ident_h,
        })
    return in_maps


def kernel(**inputs):
    global LAST_RESULTS
    if "nc" not in _CACHE:
        _CACHE["nc"] = _build()
    nc = _CACHE["nc"]
    in_maps = _host_prep(**inputs)
    trace = bool(int(os.environ.get("KERNEL_TRACE", "0")))
    res = bass_utils.run_bass_kernel_spmd(
        nc, in_maps, core_ids=list(range(8)), trace=trace)
    LAST_RESULTS = res
    out = np.empty((B, S, H), np.float32)
    for b in range(B):
        o = res.results[b]["o"]              # [P, SC, H]
        out[b] = o.transpose(1, 0, 2).reshape(S, H)
    return out



# revision 9
# speedup vs baseline: 1.6489x; 1.6489x over previous
"""CosyVoice2 attention (B=8, S=2048, H=896, 14Q/2KV GQA, RoPE, causal) as a
Trainium2 Bass/Tile kernel, data-parallel over batch across 8 NeuronCores.

v2: all-bf16 matmul datapath (fp32 PSUM accumulation).  fp32r "HIGH" mode
matmuls power-throttled the PE to ~1.2GHz for >50% of the baseline run;
bf16 halves PE energy/MAC and LDWEIGHTS cost.  Max rel err measured
3.9e-3 in numpy simulation (tolerance 2e-2).

Per-core program (one batch element per core, no collectives):
  - host supplies X^T [896, 2048] bf16 (hidden on partitions, 7 chunks of
    128), weights in matmul-ready bf16 layouts, RoPE cos/sin tables (bf16)
    with the rotate-half sign folded in, and per-partition bias columns.
  - Wq columns are host-permuted so q head-pair p lands in chunk p with
    head p on partitions 0-63 (kv0 side) and head p+7 on 64-127 (kv1):
    score matmuls then always align with the single K^T copy (no swapped
    kt2 needed).  Wo rows use the same permutation.
  - biases ride the PSUM eviction: rope does
      t1 = (psum + b)·cos4;  qe = (psum + b)·sinm4;  dst = t1 + Pneg@qe
    via two scalar_tensor_tensor ops, so no K=1 bias matmuls.
  - scores S^T[k, q] per (head-pair, k-chunk): two K=64 bf16 matmuls on
    opposite PE row-halves (tile_position) stream concurrently.
  - softmax: one ACT exp per k-chunk over the 2-bank PSUM pair, scale=1/8
    and constant -4 bias folded in; probs emitted bf16.  On diagonal
    chunks only the causally-valid column range [c0:512] is computed
    (scores, exp, attnV) and a gpsimd affine_select masks just the
    128-wide triangle band [c0:c0+128].
  - attention inner loop is software-pipelined: scores/exp of chunk kc+1
    are emitted before attnV of chunk kc so the PE never sits behind the
    ACT exp; denominator rides as a 65th ones-row in the attnV lhsT.
  - normalize: reciprocal of the denom row -> K=1 f32r matmul broadcast
    -> DVE multiply writing the A^T slab in bf16.
  - o_proj in bf16; projection/o_proj generator steps are interleaved
    into the attention chunk stream to fill PE slack under the exp.
"""

import os
import sys

for _p in ("/opt/trn_rl_repo", "/root/.axon_site/_ro/trn_rl_repo"):
    if _p not in sys.path and os.path.isdir(_p):
        sys.path.append(_p)

import contextlib

import numpy as np
import ml_dtypes

import concourse.bacc as bacc
import concourse.mybir as mybir
import concourse.tile as tile
from concourse import bass_utils

B = 8
S = 2048
H = 896
NQ = 14
NKV = 2
D = 64
THETA = 1000000.0
P = 128
HC = H // P          # 7 hidden chunks
QT = 512             # q-tile width
NQT = S // QT        # 4 q-tiles
SC = S // P          # 16 seq chunks of 128
F32 = mybir.dt.float32
F32R = mybir.dt.float32r
BF16 = mybir.dt.bfloat16
ADD = mybir.AluOpType.add
MULT = mybir.AluOpType.mult

_CACHE = {}
LAST_RESULTS = None


def _build():
    nc = bacc.Bacc("TRN2", target_bir_lowering=False, debug=False, num_devices=8)

    xt_d = nc.dram_tensor("xt", [P, HC, S], BF16, kind="ExternalInput").ap()
    wq_d = nc.dram_tensor("wq", [P, HC, H], BF16, kind="ExternalInput").ap()
    wk_d = nc.dram_tensor("wk", [P, HC, P], BF16, kind="ExternalInput").ap()
    wv_d = nc.dram_tensor("wv", [P, HC, P], BF16, kind="ExternalInput").ap()
    wo_d = nc.dram_tensor("wo", [P, HC, H], BF16, kind="ExternalInput").ap()
    bq_d = nc.dram_tensor("bqc", [P, HC], F32, kind="ExternalInput").ap()
    bk_d = nc.dram_tensor("bkc", [P, 1], F32, kind="ExternalInput").ap()
    bv_d = nc.dram_tensor("bvc", [P, 1], F32, kind="ExternalInput").ap()
    cos_d = nc.dram_tensor("cos4", [P, S], BF16, kind="ExternalInput").ap()
    sin_d = nc.dram_tensor("sinm4", [P, S], BF16, kind="ExternalInput").ap()
    pneg_d = nc.dram_tensor("pneg", [P, P], BF16, kind="ExternalInput").ap()
    ident_d = nc.dram_tensor("ident", [P, P], BF16, kind="ExternalInput").ap()
    o_d = nc.dram_tensor("o", [P, SC, H], F32, kind="ExternalOutput").ap()

    with tile.TileContext(nc) as tc, contextlib.ExitStack() as ctx:
        const = ctx.enter_context(tc.tile_pool(name="const", bufs=1))
        work = ctx.enter_context(tc.tile_pool(name="work", bufs=2))
        ppool = ctx.enter_context(tc.tile_pool(name="ppool", bufs=3))
        rpool = ctx.enter_context(tc.tile_pool(name="rpool", bufs=2))
        npool = ctx.enter_context(tc.tile_pool(name="npool", bufs=3))
        psc = ctx.enter_context(tc.tile_pool(name="psc", bufs=2, space="PSUM"))
        pssc = ctx.enter_context(tc.tile_pool(name="pssc", bufs=2, space="PSUM"))
        psav = ctx.enter_context(tc.tile_pool(name="psav", bufs=1, space="PSUM"))

        # ---- resident constants ----
        wq_sb = const.tile([P, HC, H], BF16)
        wk_sb = const.tile([P, HC, P], BF16)
        wv_sb = const.tile([P, HC, P], BF16)
        wo_sb = const.tile([P, HC, H], BF16)
        bq_sb = const.tile([P, HC], F32)
        bk_sb = const.tile([P, 1], F32)
        bv_sb = const.tile([P, 1], F32)
        cos_sb = const.tile([P, S], BF16)
        sin_sb = const.tile([P, S], BF16)
        pneg_sb = const.tile([P, P], BF16)
        ident_sb = const.tile([P, P], BF16)
        bias_exp = const.tile([P, 1], F32)
        ones64 = const.tile([1, D], BF16)
        for dst, src in ((wq_sb, wq_d), (wk_sb, wk_d), (wv_sb, wv_d),
                         (wo_sb, wo_d), (bq_sb, bq_d), (bk_sb, bk_d),
                         (bv_sb, bv_d), (cos_sb, cos_d), (sin_sb, sin_d),
                         (pneg_sb, pneg_d), (ident_sb, ident_d)):
            nc.sync.dma_start(out=dst, in_=src)
        nc.vector.memset(bias_exp, -4.0)
        nc.vector.memset(ones64, 1.0)

        # K^T resident and V' resident
        kt = const.tile([P, S], BF16)    # parts 0-63 = kv0, 64-127 = kv1
        vp = const.tile([P, SC, 130], BF16)  # [Vkv0 | ones | Vkv1 | ones]
        nc.vector.memset(vp[:, :, 64:65], 1.0)
        nc.vector.memset(vp[:, :, 129:130], 1.0)

        # absorb weight-DMA waits with tiny PE touches so they don't land
        # on the first real matmuls
        tch = psc.tile([1, 2], F32, tag="proj", name="tch")
        for t in (wq_sb, wk_sb, wv_sb, wo_sb, pneg_sb, ident_sb,
                  cos_sb, sin_sb):
            ap = (t[0:1, 0, 0:2] if len(t.shape) == 3 else t[0:1, 0:2]).bitcast(F32)
            nc.tensor.matmul(tch[:, 0:1], ap, ap, start=True, stop=True)
        for t in (bq_sb, bk_sb, bv_sb):
            ap = t[0:1, 0:1]
            nc.tensor.matmul(tch[:, 0:1], ap, ap, start=True, stop=True)

        def rope_into(dst_ap, src_psum, bias_col, cos_t, sin_t, nm):
            """dst = (src+b)*cos4 + Pneg @ ((src+b)*sinm4); 2 DVE reads"""
            t1 = rpool.tile([P, QT], F32, tag="t1", name=f"t1_{nm}")
            nc.vector.scalar_tensor_tensor(t1, src_psum, bias_col, cos_t,
                                           op0=ADD, op1=MULT)
            qe = rpool.tile([P, QT], BF16, tag="qe", name=f"qe_{nm}")
            nc.vector.scalar_tensor_tensor(qe, src_psum, bias_col, sin_t,
                                           op0=ADD, op1=MULT)
            rp = psc.tile([P, QT], F32, tag="proj", name=f"rp_{nm}")
            nc.tensor.matmul(rp, pneg_sb, qe, start=True, stop=True)
            nc.vector.tensor_add(dst_ap, t1, rp)

        state = {}

        def gen_proj(t):
            # NOTE: shared-tag PSUM tiles must be fully consumed before any
            # yield (interleaved aux steps allocate the same tag's buffers).
            tslice = slice(t * QT, (t + 1) * QT)
            xs = work.tile([P, HC, QT], BF16, tag="xs", name=f"xs{t}")
            nc.sync.dma_start(out=xs, in_=xt_d[:, :, tslice])
            cos_t = cos_sb[:, tslice]
            sin_t = sin_sb[:, tslice]
            qs = work.tile([P, HC, QT], BF16, tag="qs", name=f"qs{t}")
            state[t] = {"qs": qs}
            # K projection + rope
            kp = psc.tile([P, QT], F32, tag="proj", name=f"kp{t}")
            for c in range(HC):
                nc.tensor.matmul(kp, wk_sb[:, c, :], xs[:, c, :],
                                 start=(c == 0), stop=(c == HC - 1))
            rope_into(kt[:, tslice], kp, bk_sb, cos_t, sin_t, f"k{t}")
            yield
            # V projection: V^T (+bias) then PE-transpose per 128-chunk
            vtp = psc.tile([P, QT], F32, tag="proj", name=f"vtp{t}")
            for c in range(HC):
                nc.tensor.matmul(vtp, wv_sb[:, c, :], xs[:, c, :],
                                 start=(c == 0), stop=(c == HC - 1))
            vt_sb = rpool.tile([P, QT], BF16, tag="vt_sb", name=f"vt{t}")
            nc.vector.tensor_scalar_add(vt_sb, vtp, bv_sb)
            for j in range(4):
                sc_i = t * 4 + j
                vtr = psc.tile([P, P], BF16, tag="proj", name=f"vtr{sc_i}")
                nc.tensor.transpose(vtr, vt_sb[:, j * P:(j + 1) * P], ident_sb)
                nc.vector.tensor_copy(vp[:, sc_i, 0:64], vtr[:, 0:64])
                nc.vector.tensor_copy(vp[:, sc_i, 65:129], vtr[:, 64:128])
            yield
            # Q projection + rope (7 head-pair chunks)
            for c in range(HC):
                qp = psc.tile([P, QT], F32, tag="proj", name=f"qp{t}_{c}")
                for hcc in range(HC):
                    nc.tensor.matmul(qp, wq_sb[:, hcc, c * P:(c + 1) * P],
                                     xs[:, hcc, :],
                                     start=(hcc == 0), stop=(hcc == HC - 1))
                rope_into(qs[:, c, :], qp, bq_sb[:, c:c + 1],
                          cos_t, sin_t, f"q{t}_{c}")
                yield

        def gen_oproj(t):
            aslab = state[t]["aslab"]
            for j in range(4):
                sc_i = t * 4 + j
                jsl = slice(j * P, (j + 1) * P)
                for n0, nw in ((0, 512), (512, 384)):
                    op = psc.tile([P, 512], F32, tag="proj",
                                  name=f"op{sc_i}_{n0}")
                    for c in range(HC):
                        nc.tensor.matmul(op[:, 0:nw], aslab[:, c, jsl],
                                         wo_sb[:, c, n0:n0 + nw],
                                         start=(c == 0), stop=(c == HC - 1))
                    osb = npool.tile([P, 512], F32, tag="osb", bufs=2,
                                     name=f"os{sc_i}_{n0}")
                    nc.vector.tensor_copy(osb[:, 0:nw], op[:, 0:nw])
                    nc.sync.dma_start(out=o_d[:, sc_i, n0:n0 + nw],
                                      in_=osb[:, 0:nw])
                    yield

        def gen_attention_pair(t, hp):
            qs = state[t]["qs"]
            aslab = state[t]["aslab"]
            nkc = (t + 1) * 4
            av0 = psav.tile([65, QT], F32, tag="av0", name=f"av0_{t}_{hp}")
            av1 = psav.tile([65, QT], F32, tag="av1", name=f"av1_{t}_{hp}")

            def emit_av(kc, probs, c0):
                csl = slice(c0, QT)
                nc.tensor.matmul(av0[:, csl], vp[:, kc, 0:65],
                                 probs[:, 0, csl], start=(kc == 0),
                                 stop=(kc == nkc - 1), skip_group_check=True)
                nc.tensor.matmul(av1[:, csl], vp[:, kc, 65:130],
                                 probs[:, 1, csl], start=(kc == 0),
                                 stop=(kc == nkc - 1), skip_group_check=True)

            pending = None
            for kc in range(nkc):
                c0 = (kc - 4 * t) * P if kc >= 4 * t else 0
                ksl = slice(kc * P, (kc + 1) * P)
                csl = slice(c0, QT)
                st = pssc.tile([P, 2, QT], F32, tag="st",
                               name=f"st{t}_{hp}_{kc}")
                nc.tensor.matmul(st[:, 0, csl], kt[0:64, ksl],
                                 qs[0:64, hp, csl], start=True, stop=True)
                nc.tensor.matmul(st[:, 1, csl], kt[64:128, ksl],
                                 qs[64:128, hp, csl], start=True, stop=True,
                                 tile_position=(64, 0))
                probs = ppool.tile([P, 2, QT], BF16, tag="probs",
                                   name=f"pr{t}_{hp}_{kc}")
                nc.scalar.activation(probs[:, :, csl], st[:, :, csl],
                                     mybir.ActivationFunctionType.Exp,
                                     bias=bias_exp, scale=0.125)
                if kc >= 4 * t:  # diagonal chunk: mask the triangle band
                    nc.gpsimd.affine_select(
                        out=probs[:, :, c0:c0 + P],
                        in_=probs[:, :, c0:c0 + P],
                        pattern=[[0, 2], [1, P]],
                        compare_op=mybir.AluOpType.is_ge, fill=0.0,
                        base=0, channel_multiplier=-1)
                if pending is not None:
                    emit_av(*pending)
                pending = (kc, probs, c0)
                yield
            emit_av(*pending)
            # normalize + write A^T slab
            for av, rh in ((av0, 0), (av1, 64)):
                nm = f"n{t}_{hp}_{rh}"
                dsb = npool.tile([1, QT], BF16, tag="dsb", name=f"d{nm}")
                nc.vector.tensor_copy(dsb, av[64:65, :])
                bc = psc.tile([64, QT], F32, tag="proj", name=f"bc{nm}")
                nc.tensor.matmul(bc, ones64, dsb, start=True, stop=True)
                rc = npool.tile([64, QT], F32, tag="rc", name=f"r{nm}")
                nc.vector.reciprocal_approx_fast(rc, bc)
                nc.vector.tensor_mul(aslab[rh:rh + 64, hp, :],
                                     av[0:64, :], rc)
            yield

        # ---- software-pipelined emission ----
        for _ in gen_proj(0):
            pass
        for t in range(NQT):
            state[t]["aslab"] = work.tile([P, HC, QT], BF16, tag="aslab",
                                          name=f"aslab{t}")
            aux = []
            if t + 1 < NQT:
                aux.append(gen_proj(t + 1))
            if t >= 1:
                aux.append(gen_oproj(t - 1))
            nchunks = 7 * (4 * t + 4) + 7
            naux = 9 + (8 if t >= 1 else 0)
            stride = max(1, nchunks // (naux + 1))
            step = 0
            for hp in range(7):
                for _ in gen_attention_pair(t, hp):
                    step += 1
                    if step % stride == 0 and aux:
                        g = aux[step % len(aux)]
                        try:
                            next(g)
                        except StopIteration:
                            aux.remove(g)
            for g in aux:
                for _ in g:
                    pass
        for _ in gen_oproj(NQT - 1):
            pass
    nc.compile()
    return nc


def _host_prep(hidden_states, position_ids, Wq, bq, Wk, bk, Wv, bv, Wo):
    """Build per-core input maps (host-side layout work)."""
    bf16 = ml_dtypes.bfloat16
    f32 = np.float32
    HALF = 32

    def chunked(w, dt=bf16):  # [H, N] -> [P, HC, N]
        return np.ascontiguousarray(
            w.reshape(HC, P, -1).transpose(1, 0, 2)).astype(dt)

    # q head-pair permutation: chunk p partitions 0-63 = head p (kv0),
    # 64-127 = head p+7 (kv1); Wo rows use the same ordering.
    perm = np.empty(H, np.int64)
    for hp in range(HC):
        for p in range(P):
            h = hp if p < 64 else hp + 7
            perm[hp * P + p] = h * D + (p % 64)
    wq_h = chunked(Wq[:, perm])
    wk_h = chunked(Wk)
    wv_h = chunked(Wv)
    wo_h = chunked(Wo[perm])
    bq_h = np.ascontiguousarray(bq[perm].reshape(HC, P).T).astype(f32)
    bk_h = bk.reshape(P, 1).astype(f32)
    bv_h = bv.reshape(P, 1).astype(f32)
    pneg = np.zeros((P, P), f32)
    for i in range(P):
        pneg[i, i ^ 32] = -1.0
    pneg_h = pneg.astype(bf16)
    ident_h = np.eye(P, dtype=f32).astype(bf16)

    inv_freq = (1.0 / (THETA ** (np.arange(0, HALF, dtype=np.float64) / HALF)))
    pidx = np.arange(P)
    sign = np.where((pidx % 64) >= HALF, 1.0, -1.0)[:, None]

    in_maps = []
    for b in range(B):
        xt = np.ascontiguousarray(
            hidden_states[b].T.reshape(HC, P, S).transpose(1, 0, 2)).astype(bf16)
        ang = position_ids[b].astype(np.float64)[None, :] * \
            inv_freq[pidx % HALF][:, None]          # [P, S]
        cos4 = np.cos(ang).astype(bf16)
        sinm4 = (np.sin(ang) * sign).astype(bf16)
        in_maps.append({
            "xt": xt, "wq": wq_h, "wk": wk_h, "wv": wv_h, "wo": wo_h,
            "bqc": bq_h, "bkc": bk_h, "bvc": bv_h,
            "cos4": cos4, "sinm4": sinm4, "pneg": pneg_h, "ident": ident_h,
        })
    return in_maps


def kernel(**inputs):
    global LAST_RESULTS
    if "nc" not in _CACHE:
        _CACHE["nc"] = _build()
    nc = _CACHE["nc"]
    in_maps = _host_prep(**inputs)
    trace = bool(int(os.environ.get("KERNEL_TRACE", "0")))
    res = bass_utils.run_bass_kernel_spmd(
        nc, in_maps, core_ids=list(range(8)), trace=trace)
    LAST_RESULTS = res
    out = np.empty((B, S, H), np.float32)
    for b in range(B):
        o = res.results[b]["o"]              # [P, SC, H]
        out[b] = o.transpose(1, 0, 2).reshape(S, H)
    return out
